# revision 13
# baseline (speedup 1.0000x reference)
"""DiffAttention TRN2 kernel: 8-way (batch x seq-half) sharded, zero collectives.

v2 design (vs baseline): bf16 dataflow end-to-end, 3-slot rotating PSUM S
buffer with paired 2048-wide exp instructions, combine phase entirely on
ACT/DVE/GpSimd (ln/exp algebra instead of DVE reciprocals, GpSimd partition
broadcast/reduce instead of tensor-engine row broadcasts), combine deferred
by one head so the in-order tensor queue never stalls on it, single
activation-table set (exp/ln/square) pinned for the whole kernel.

Shapes: x [4, 4096, 1024], H=16 heads, (h, 2 branches, 32 dims) head layout,
v head dim 64. Each core: one (batch, query-half): 2048 queries x 4096 keys.

  - qkv phase: Q^T [1024,2048], K^T [1024,4096] (c on partitions), V
    [4096,1024] (tokens on partitions) -> DRAM scratch in bf16.
  - attention per (head, branch, q-chunk of 1024): S^T kt-tiles [128k, 1024q]
    in a manually-rotated [128,3,1024] PSUM tile; exp over kt-PAIRS (2048
    wide) -> bf16 es; PV accumulates O^T [65,1024] (65th row = softmax
    denominator via ones column of V_aug).
  - combine per head (deferred one head): t = lam*exp(ln Z1 - ln Z2) row,
    GpSimd-broadcast; od = O1 - t*O2; rms via GpSimd partition_all_reduce of
    [od^2; VD*eps*Z1^2] and rsqrt = exp(-0.5*ln u); write ot_acc bf16.
  - proj: lhsT = ot_acc bf16 tiles, wproj converted to bf16, bias via K=1.
"""

import sys

import numpy as np

for p in ("/opt/trn_rl_repo",):
    if p not in sys.path:
        sys.path.insert(0, p)

import concourse.bacc as bacc_mod
import concourse.bass_isa as bass_isa
import concourse.mybir as mybir
from concourse.bass_utils import run_bass_kernel_spmd
from concourse.hw_specs import get_activation_tables
from concourse.tile import TileContext

F32 = mybir.dt.float32
F32R = mybir.dt.float32r
BF16 = mybir.dt.bfloat16
AF = mybir.ActivationFunctionType

B, N, DIM, H, HD = 4, 4096, 1024, 16, 32
VD = 2 * HD  # 64, per-head v dim
NQ = 2048  # query rows per core
NCORES = 8
LAMBDA_INIT = 0.2
EPS = 1e-5
SCALE = HD ** -0.5
KT = N // 128  # 32 key tiles
CIN = DIM // 128  # 8 contraction tiles
QW = 1024  # q chunk width in attention

_CACHE = {}

_ACT_SET = "natural_log_exp_and_others"
_ACT_FUNCS = {AF.Exp, AF.Ln, AF.Square}


class _Bacc(bacc_mod.Bacc):
    """Bacc that pins exp/ln/square to one activation-table set so the
    table is loaded once instead of thrashing between per-func sets."""

    def insert_act_table_loads(self):
        has_activation = any(
            isinstance(i, mybir.InstActivation)
            for b in self.main_func.blocks
            for i in b.instructions
        )
        if not has_activation:
            return
        tables = []
        for name, funcs in get_activation_tables(self.m.arch).items():
            if name == _ACT_SET:
                tables.append((name, funcs))
            else:
                tables.append((name, funcs - _ACT_FUNCS))
        bacc_mod._bass_rust.insert_act_table_loads(self, tables)


def _r(ap):
    return ap.bitcast(F32R)


def build_nc(lam: float):
    nc = _Bacc(None, target_bir_lowering=False)

    xbt = nc.declare_dram_parameter("xbt", [DIM, N], F32, isOutput=False)
    wqkvt = nc.declare_dram_parameter("wqkvt", [DIM, 3 * DIM], F32, isOutput=False)
    wprojt = nc.declare_dram_parameter("wprojt", [DIM, DIM], F32, isOutput=False)
    bproj = nc.declare_dram_parameter("bproj", [1, DIM], F32, isOutput=False)
    weff = nc.declare_dram_parameter("weff", [VD, 1], F32, isOutput=False)
    y = nc.declare_dram_parameter("y", [NQ, DIM], F32, isOutput=True)

    qt_s = nc.dram_tensor("qt_scratch", [DIM, NQ], BF16)
    kt_s = nc.dram_tensor("kt_scratch", [DIM, N], BF16)
    v_s = nc.dram_tensor("v_scratch", [H, 128, KT, VD], BF16)

    with nc.allow_low_precision(reason="bf16 attention within 2e-2 tolerance"), \
         TileContext(nc) as tc:
        with tc.tile_pool(name="const", bufs=1) as constp:
            weff_t = constp.tile([128, 1], F32)
            for rp in (0, VD):
                nc.sync.dma_start(out=weff_t[rp:rp + VD, :], in_=weff[:, :])
            ones1 = constp.tile([1, 128], BF16)
            nc.vector.memset(ones1, 1.0)
            bp_b = constp.tile([1, DIM], BF16)

            # ================= phase A: qkv =================
            with (
                tc.tile_pool(name="xbt_p", bufs=1) as xbtp,
                tc.tile_pool(name="wq_p", bufs=4) as wqp,
                tc.tile_pool(name="drain_p", bufs=3) as drp,
                tc.tile_pool(name="psA", bufs=2, space="PSUM") as psA,
            ):
                bpf = drp.tile([1, DIM], F32, tag="bpf")
                nc.sync.dma_start(out=bpf, in_=bproj[:, :])
                nc.vector.tensor_copy(bp_b, bpf)

                xb = xbtp.tile([128, CIN, N], F32R)
                for ch in range(4):
                    nc.sync.dma_start(
                        out=xb[:, :, ch * 1024:(ch + 1) * 1024],
                        in_=xbt[:, :].rearrange("(t p) n -> p t n", p=128)
                        [:, :, ch * 1024:(ch + 1) * 1024].bitcast(F32R),
                    )
                # --- Q^T and K^T co-tiles ---
                for co in range(2 * CIN):  # 0..7 Q, 8..15 K
                    is_q = co < CIN
                    tok = NQ if is_q else N
                    for ch in range(tok // 1024):
                        ps = psA.tile([128, 1024], F32, tag="ps")
                        for ci in range(CIN):
                            wt = wqp.tile([128, 128], F32R, tag="w")
                            nc.sync.dma_start(
                                out=wt,
                                in_=wqkvt[ci * 128:(ci + 1) * 128,
                                          co * 128:(co + 1) * 128].bitcast(F32R),
                            )
                            for sb in range(2):
                                nc.tensor.matmul(
                                    ps[:, sb * 512:(sb + 1) * 512],
                                    _r(wt),
                                    _r(xb[:, ci, ch * 1024 + sb * 512:
                                          ch * 1024 + (sb + 1) * 512]),
                                    start=(ci == 0),
                                    stop=(ci == CIN - 1),
                                )
                        dr = drp.tile([128, 1024], BF16, tag="dr")
                        nc.vector.tensor_copy(dr, ps)
                        dst = qt_s if is_q else kt_s
                        coo = co if is_q else co - CIN
                        nc.sync.dma_start(
                            out=dst[coo * 128:(coo + 1) * 128,
                                    ch * 1024:(ch + 1) * 1024],
                            in_=dr,
                        )
                # --- V in c-chunks of 512, stored head-major for fast staging ---
                with tc.tile_pool(name="wv_p", bufs=8) as wvp:
                    for cc in range(DIM // 512):
                        wv_tiles = []
                        for ci in range(CIN):
                            wv = wvp.tile([128, 512], F32R, tag="wv")
                            nc.sync.dma_start(
                                out=wv,
                                in_=wqkvt[ci * 128:(ci + 1) * 128,
                                          2 * DIM + cc * 512:
                                          2 * DIM + (cc + 1) * 512].bitcast(F32R),
                            )
                            wv_tiles.append(wv)
                        for kt in range(KT):
                            psv = psA.tile([128, 512], F32, tag="ps")
                            for ci in range(CIN):
                                nc.tensor.matmul(
                                    psv,
                                    _r(xb[:, ci, kt * 128:(kt + 1) * 128]),
                                    _r(wv_tiles[ci]),
                                    start=(ci == 0),
                                    stop=(ci == CIN - 1),
                                )
                            drv = drp.tile([128, 512], BF16, tag="drv")
                            nc.vector.tensor_copy(drv, psv)
                            nc.sync.dma_start(
                                out=v_s[cc * 8:(cc + 1) * 8, :, kt, :]
                                .rearrange("h p c -> p h c"),
                                in_=drv.rearrange("p (h c) -> p h c", h=8),
                            )

            # ================= phase B: attention =================
            with (
                tc.tile_pool(name="ot", bufs=1) as otp,
                tc.tile_pool(name="wpb_p", bufs=1) as wpbp,
            ):
              with (
                tc.tile_pool(name="stage", bufs=1) as stp,
                tc.tile_pool(name="es_p", bufs=1) as esp,
                tc.tile_pool(name="osb_p", bufs=1) as osbp,
                tc.tile_pool(name="row_p", bufs=1) as rowp,
                tc.tile_pool(name="cmb_p", bufs=1) as cmbp,
              ):
                ot_acc = otp.tile([128, CIN, NQ], BF16)
                wp_b = wpbp.tile([128, CIN, DIM], BF16)

                def emit_combine(h, osb, zrs):
                    p0 = (h % 2) * VD
                    for qc in range(2):
                        zr1, zr2 = zrs[qc]
                        o1 = osb[(0, qc)]
                        o2 = osb[(1, qc)]
                        l1 = rowp.tile([1, QW], F32, tag="l1", bufs=1)
                        nc.scalar.activation(l1, zr1, AF.Ln)
                        l2 = rowp.tile([1, QW], F32, tag="l2", bufs=1)
                        nc.scalar.activation(l2, zr2, AF.Ln)
                        drow = rowp.tile([1, QW], F32, tag="drow", bufs=1)
                        nc.vector.tensor_sub(drow, l1, l2)
                        tr = rowp.tile([1, QW], F32, tag="tr", bufs=1)
                        nc.scalar.activation(tr, drow, AF.Exp)  # Z1/Z2
                        tr2 = rowp.tile([1, QW], BF16, tag="tr2", bufs=1)
                        nc.vector.tensor_scalar_mul(tr2, tr, lam)
                        tb = cmbp.tile([VD, QW], BF16, tag="tb", bufs=1)
                        nc.gpsimd.partition_broadcast(tb, tr2, channels=VD)
                        tmp = cmbp.tile([VD, QW], BF16, tag="tmp", bufs=1)
                        nc.vector.tensor_mul(tmp, tb, o2)
                        od = cmbp.tile([VD, QW], BF16, tag="od", bufs=1)
                        nc.vector.tensor_sub(od, o1, tmp)
                        squ = cmbp.tile([VD + 1, QW], BF16, tag="squ", bufs=1)
                        nc.vector.tensor_mul(squ[0:VD, :], od, od)
                        ez = rowp.tile([1, QW], F32, tag="ez", bufs=1)
                        nc.vector.tensor_mul(ez, zr1, zr1)
                        nc.vector.tensor_scalar_mul(
                            squ[VD:VD + 1, :], ez, float(VD) * EPS
                        )
                        u = cmbp.tile([VD + 1, QW], F32, tag="u", bufs=1)
                        nc.gpsimd.partition_all_reduce(
                            u, squ, VD + 1, bass_isa.ReduceOp.add
                        )
                        lu = cmbp.tile([VD, QW], F32, tag="lu", bufs=1)
                        nc.scalar.activation(lu, u[0:VD, :], AF.Ln)
                        rr = cmbp.tile([VD, QW], BF16, tag="rr", bufs=1)
                        nc.scalar.activation(rr, lu, AF.Exp, scale=-0.5)
                        on2 = cmbp.tile([VD, QW], BF16, tag="on2", bufs=1)
                        nc.vector.tensor_mul(on2, od, rr)
                        nc.vector.tensor_scalar_mul(
                            ot_acc[p0:p0 + VD, h // 2, qc * QW:(qc + 1) * QW],
                            on2,
                            weff_t[p0:p0 + VD, :],
                        )

                with (
                    tc.tile_pool(name="Tp", bufs=1, space="PSUM") as Tp,
                    tc.tile_pool(name="psO", bufs=1, space="PSUM") as psOp,
                ):
                    T = Tp.tile([128, 3, 1024], F32)
                    slot_ctr = [0]

                    def attn_unit(qh, kh, vh, qc):
                        o_ps = psOp.tile([65, QW], F32, tag="o")
                        es_info = {}
                        prev = None

                        def emit_pv(kts):
                            for kt in kts:
                                es, half = es_info[kt]
                                for sbi in range(2):
                                    nc.tensor.matmul(
                                        o_ps[:, sbi * 512:(sbi + 1) * 512],
                                        vh[:, kt, 0:VD + 1],
                                        es[:, half, sbi * 512:(sbi + 1) * 512],
                                        start=(kt == 0),
                                        stop=(kt == KT - 1),
                                    )

                        for m in range(KT // 2):
                            cur = []
                            for i in (0, 1):
                                kt = 2 * m + i
                                s = slot_ctr[0]
                                slot_ctr[0] = (s + 1) % 3
                                kb = 64 * (kt % 2)
                                klhs = kh[kb:kb + HD, kt // 2, :]
                                for sbi in range(2):
                                    nc.tensor.matmul(
                                        T[:, s, sbi * 512:(sbi + 1) * 512],
                                        klhs,
                                        qh[kb:kb + HD,
                                           qc * QW + sbi * 512:
                                           qc * QW + (sbi + 1) * 512],
                                        start=True,
                                        stop=True,
                                    )
                                cur.append((kt, s))
                            if prev is not None:
                                emit_pv(prev)
                            (ka, sa), (kb2, sb2) = cur
                            lo, hi = (sa, sb2) if sa < sb2 else (sb2, sa)
                            es = esp.tile([128, 2, 1024], BF16, tag="es", bufs=3)
                            nc.scalar.activation(
                                es, T[:, lo:hi + 1:(hi - lo), :], AF.Exp,
                                scale=SCALE,
                            )
                            es_info[ka] = (es, 0 if sa == lo else 1)
                            es_info[kb2] = (es, 0 if sb2 == lo else 1)
                            prev = (ka, kb2)
                        emit_pv(prev)
                        return o_ps

                    def stage_head(h):
                        vh = stp.tile([128, KT, VD + 2], BF16, tag="vh", bufs=2)
                        nc.sync.dma_start(out=vh[:, :, 0:VD], in_=v_s[h])
                        nc.vector.memset(vh[:, :, VD:VD + 1], 1.0)
                        qks = []
                        for br in range(2):
                            r0 = h * VD + br * HD
                            qh = stp.tile([128, NQ], BF16, tag="qh", bufs=4)
                            for rp in (0, 64):
                                nc.sync.dma_start(
                                    out=qh[rp:rp + HD, :], in_=qt_s[r0:r0 + HD, :]
                                )
                            kh = stp.tile([128, KT // 2, 128], BF16, tag="kh",
                                          bufs=4)
                            for bq in range(2):
                                nc.sync.dma_start(
                                    out=kh[bq * 64:bq * 64 + HD, :, :],
                                    in_=kt_s[r0:r0 + HD, :].rearrange(
                                        "d (g b t) -> d g b t", b=2, t=128
                                    )[:, :, bq, :],
                                )
                            qks.append((qh, kh))
                        return vh, qks

                    # wproj load+convert overlaps head-0 staging/attention
                    for ci in range(CIN):
                        wpf = stp.tile([128, DIM], F32, tag="wpf", bufs=1)
                        nc.sync.dma_start(
                            out=wpf, in_=wprojt[ci * 128:(ci + 1) * 128, :]
                        )
                        nc.vector.tensor_copy(wp_b[:, ci, :], wpf)

                    prev_head = None
                    staged = stage_head(0)
                    for h in range(H):
                        nxt = stage_head(h + 1) if h + 1 < H else None
                        vh, qks = staged
                        osb = {}
                        zrs = {}
                        for br in range(2):
                            qh, kh = qks[br]
                            for qc in range(2):
                                o_ps = attn_unit(qh, kh, vh, qc)
                                o_sb = osbp.tile([VD, QW], BF16, tag="osb",
                                                 bufs=8)
                                nc.vector.tensor_copy(o_sb, o_ps[0:VD, :])
                                zt = osbp.tile([1, QW], F32, tag=f"zr{br}",
                                               bufs=4)
                                nc.vector.tensor_copy(zt, o_ps[VD:VD + 1, :])
                                if br == 0:
                                    zrs[qc] = [zt, None]
                                else:
                                    zrs[qc][1] = zt
                                osb[(br, qc)] = o_sb
                        if prev_head is not None:
                            emit_combine(*prev_head)
                        prev_head = (h, osb, zrs)
                        staged = nxt
                    emit_combine(*prev_head)

              # ================= phase C: proj =================
              with (
                  tc.tile_pool(name="psY", bufs=2, space="PSUM") as psY,
                  tc.tile_pool(name="yd_p", bufs=3) as ydp,
              ):
                if True:
                    for qt in range(NQ // 128):
                        yps = psY.tile([128, 1024], F32, tag="y")
                        for sbi in range(2):
                            for ci in range(CIN):
                                nc.tensor.matmul(
                                    yps[:, sbi * 512:(sbi + 1) * 512],
                                    ot_acc[:, ci, qt * 128:(qt + 1) * 128],
                                    wp_b[:, ci, sbi * 512:(sbi + 1) * 512],
                                    start=(ci == 0),
                                    stop=False,
                                )
                            nc.tensor.matmul(
                                yps[:, sbi * 512:(sbi + 1) * 512],
                                ones1,
                                bp_b[:, sbi * 512:(sbi + 1) * 512],
                                start=False,
                                stop=True,
                            )
                        yd = ydp.tile([128, 1024], F32, tag="yd")
                        nc.vector.tensor_copy(yd, yps)
                        nc.sync.dma_start(
                            out=y[qt * 128:(qt + 1) * 128, :], in_=yd
                        )
    nc.finalize()
    return nc


def make_in_maps(inputs):
    x = np.asarray(inputs["x"], np.float32)
    wqkvt = np.ascontiguousarray(np.asarray(inputs["w_qkv"], np.float32).T)
    wprojt = np.ascontiguousarray(np.asarray(inputs["w_proj"], np.float32).T)
    bp = np.asarray(inputs["b_proj"], np.float32).reshape(1, DIM)
    weff = (np.asarray(inputs["sub_norm_w"], np.float32)
            * (1.0 - LAMBDA_INIT) * float(np.sqrt(VD))).reshape(VD, 1)
    in_maps = []
    for c in range(NCORES):
        b, half = c // 2, c % 2
        xt = np.asarray(x[b].T)  # [DIM, N]
        if half == 1:  # query rows first
            xt = np.concatenate([xt[:, NQ:], xt[:, :NQ]], axis=1)
        in_maps.append({
            "xbt": np.ascontiguousarray(xt),
            "wqkvt": wqkvt,
            "wprojt": wprojt,
            "bproj": bp,
            "weff": weff,
        })
    return in_maps


def kernel(x, w_qkv, w_proj, b_proj, lambda_q1, lambda_k1, lambda_q2,
           lambda_k2, sub_norm_w):
    lam = float(
        np.exp(np.sum(np.float64(lambda_q1) * np.float64(lambda_k1)))
        - np.exp(np.sum(np.float64(lambda_q2) * np.float64(lambda_k2)))
        + LAMBDA_INIT
    )

    key = round(lam, 12)
    if key not in _CACHE:
        _CACHE[key] = build_nc(lam)
    nc = _CACHE[key]

    in_maps = make_in_maps(dict(
        x=x, w_qkv=w_qkv, w_proj=w_proj, b_proj=b_proj, sub_norm_w=sub_norm_w
    ))
    res = run_bass_kernel_spmd(nc, in_maps, list(range(NCORES)))
    out = np.empty((B, N, DIM), np.float32)
    for c in range(NCORES):
        b, half = c // 2, c % 2
        out[b, half * NQ:(half + 1) * NQ, :] = res.results[c]["y"]
    return out


# revision 15
# speedup vs baseline: 1.0215x; 1.0215x over previous
"""DiffAttention TRN2 kernel: 8-way (batch x seq-half) sharded, zero collectives.

v2 design (vs baseline): bf16 dataflow end-to-end, 3-slot rotating PSUM S
buffer with paired 2048-wide exp instructions, combine phase entirely on
ACT/DVE/GpSimd (ln/exp algebra instead of DVE reciprocals, GpSimd partition
broadcast/reduce instead of tensor-engine row broadcasts), combine deferred
by one head so the in-order tensor queue never stalls on it, single
activation-table set (exp/ln/square) pinned for the whole kernel.

Shapes: x [4, 4096, 1024], H=16 heads, (h, 2 branches, 32 dims) head layout,
v head dim 64. Each core: one (batch, query-half): 2048 queries x 4096 keys.

  - qkv phase: Q^T [1024,2048], K^T [1024,4096] (c on partitions), V
    [4096,1024] (tokens on partitions) -> DRAM scratch in bf16.
  - attention per (head, branch, q-chunk of 1024): S^T kt-tiles [128k, 1024q]
    in a manually-rotated [128,3,1024] PSUM tile; exp over kt-PAIRS (2048
    wide) -> bf16 es; PV accumulates O^T [65,1024] (65th row = softmax
    denominator via ones column of V_aug).
  - combine per head (deferred one head): t = lam*exp(ln Z1 - ln Z2) row,
    GpSimd-broadcast; od = O1 - t*O2; rms via GpSimd partition_all_reduce of
    [od^2; VD*eps*Z1^2] and rsqrt = exp(-0.5*ln u); write ot_acc bf16.
  - proj: lhsT = ot_acc bf16 tiles, wproj converted to bf16, bias via K=1.
"""

import sys

import numpy as np

for p in ("/opt/trn_rl_repo",):
    if p not in sys.path:
        sys.path.insert(0, p)

import concourse.bacc as bacc_mod
import concourse.bass_isa as bass_isa
import concourse.mybir as mybir
from concourse.bass_utils import run_bass_kernel_spmd
from concourse.hw_specs import get_activation_tables
from concourse.tile import TileContext

F32 = mybir.dt.float32
F32R = mybir.dt.float32r
BF16 = mybir.dt.bfloat16
AF = mybir.ActivationFunctionType

B, N, DIM, H, HD = 4, 4096, 1024, 16, 32
VD = 2 * HD  # 64, per-head v dim
NQ = 2048  # query rows per core
NCORES = 8
LAMBDA_INIT = 0.2
EPS = 1e-5
SCALE = HD ** -0.5
KT = N // 128  # 32 key tiles
CIN = DIM // 128  # 8 contraction tiles
QW = 1024  # q chunk width in attention

_CACHE = {}

_ACT_SET = "natural_log_exp_and_others"
_ACT_FUNCS = {AF.Exp, AF.Ln, AF.Square}


class _Bacc(bacc_mod.Bacc):
    """Bacc that pins exp/ln/square to one activation-table set so the
    table is loaded once instead of thrashing between per-func sets."""

    def insert_act_table_loads(self):
        has_activation = any(
            isinstance(i, mybir.InstActivation)
            for b in self.main_func.blocks
            for i in b.instructions
        )
        if not has_activation:
            return
        tables = []
        for name, funcs in get_activation_tables(self.m.arch).items():
            if name == _ACT_SET:
                tables.append((name, funcs))
            else:
                tables.append((name, funcs - _ACT_FUNCS))
        bacc_mod._bass_rust.insert_act_table_loads(self, tables)


def _r(ap):
    return ap.bitcast(F32R)


def build_nc(lam: float):
    nc = _Bacc(None, target_bir_lowering=False)

    xbt = nc.declare_dram_parameter("xbt", [DIM, N], F32, isOutput=False)
    wqkvt = nc.declare_dram_parameter("wqkvt", [DIM, 3 * DIM], F32, isOutput=False)
    wprojt = nc.declare_dram_parameter("wprojt", [DIM, DIM], F32, isOutput=False)
    bproj = nc.declare_dram_parameter("bproj", [1, DIM], F32, isOutput=False)
    weff = nc.declare_dram_parameter("weff", [VD, 1], F32, isOutput=False)
    y = nc.declare_dram_parameter("y", [NQ, DIM], F32, isOutput=True)

    qt_s = nc.dram_tensor("qt_scratch", [DIM, NQ], BF16)
    kt_s = nc.dram_tensor("kt_scratch", [DIM, N], BF16)
    v_s = nc.dram_tensor("v_scratch", [H, 128, KT, VD], BF16)

    with nc.allow_low_precision(reason="bf16 attention within 2e-2 tolerance"), \
         TileContext(nc) as tc:
        with tc.tile_pool(name="const", bufs=1) as constp:
            weff_t = constp.tile([128, 1], F32)
            for rp in (0, VD):
                nc.sync.dma_start(out=weff_t[rp:rp + VD, :], in_=weff[:, :])
            ones1 = constp.tile([1, 128], BF16)
            nc.vector.memset(ones1, 1.0)
            bp_b = constp.tile([1, DIM], BF16)

            # ================= phase A: qkv =================
            with (
                tc.tile_pool(name="xbt_p", bufs=1) as xbtp,
                tc.tile_pool(name="wq_p", bufs=4) as wqp,
                tc.tile_pool(name="drain_p", bufs=3) as drp,
                tc.tile_pool(name="psA", bufs=2, space="PSUM") as psA,
            ):
                bpf = drp.tile([1, DIM], F32, tag="bpf")
                nc.sync.dma_start(out=bpf, in_=bproj[:, :])
                nc.vector.tensor_copy(bp_b, bpf)

                xb = xbtp.tile([128, CIN, N], F32R)
                for ch in range(4):
                    nc.sync.dma_start(
                        out=xb[:, :, ch * 1024:(ch + 1) * 1024],
                        in_=xbt[:, :].rearrange("(t p) n -> p t n", p=128)
                        [:, :, ch * 1024:(ch + 1) * 1024].bitcast(F32R),
                    )
                # --- Q^T and K^T co-tiles ---
                for co in range(2 * CIN):  # 0..7 Q, 8..15 K
                    is_q = co < CIN
                    tok = NQ if is_q else N
                    for ch in range(tok // 1024):
                        ps = psA.tile([128, 1024], F32, tag="ps")
                        for ci in range(CIN):
                            wt = wqp.tile([128, 128], F32R, tag="w")
                            nc.sync.dma_start(
                                out=wt,
                                in_=wqkvt[ci * 128:(ci + 1) * 128,
                                          co * 128:(co + 1) * 128].bitcast(F32R),
                            )
                            for sb in range(2):
                                nc.tensor.matmul(
                                    ps[:, sb * 512:(sb + 1) * 512],
                                    _r(wt),
                                    _r(xb[:, ci, ch * 1024 + sb * 512:
                                          ch * 1024 + (sb + 1) * 512]),
                                    start=(ci == 0),
                                    stop=(ci == CIN - 1),
                                )
                        dr = drp.tile([128, 1024], BF16, tag="dr")
                        nc.vector.tensor_copy(dr, ps)
                        dst = qt_s if is_q else kt_s
                        coo = co if is_q else co - CIN
                        nc.sync.dma_start(
                            out=dst[coo * 128:(coo + 1) * 128,
                                    ch * 1024:(ch + 1) * 1024],
                            in_=dr,
                        )
                # --- V in c-chunks of 512, stored head-major for fast staging ---
                with tc.tile_pool(name="wv_p", bufs=8) as wvp:
                    for cc in range(DIM // 512):
                        wv_tiles = []
                        for ci in range(CIN):
                            wv = wvp.tile([128, 512], F32R, tag="wv")
                            nc.sync.dma_start(
                                out=wv,
                                in_=wqkvt[ci * 128:(ci + 1) * 128,
                                          2 * DIM + cc * 512:
                                          2 * DIM + (cc + 1) * 512].bitcast(F32R),
                            )
                            wv_tiles.append(wv)
                        for kt in range(KT):
                            psv = psA.tile([128, 512], F32, tag="ps")
                            for ci in range(CIN):
                                nc.tensor.matmul(
                                    psv,
                                    _r(xb[:, ci, kt * 128:(kt + 1) * 128]),
                                    _r(wv_tiles[ci]),
                                    start=(ci == 0),
                                    stop=(ci == CIN - 1),
                                )
                            drv = drp.tile([128, 512], BF16, tag="drv")
                            nc.vector.tensor_copy(drv, psv)
                            nc.sync.dma_start(
                                out=v_s[cc * 8:(cc + 1) * 8, :, kt, :]
                                .rearrange("h p c -> p h c"),
                                in_=drv.rearrange("p (h c) -> p h c", h=8),
                            )

            # ================= phase B: attention =================
            with (
                tc.tile_pool(name="ot", bufs=1) as otp,
                tc.tile_pool(name="wpb_p", bufs=1) as wpbp,
            ):
              with (
                tc.tile_pool(name="stage", bufs=1) as stp,
                tc.tile_pool(name="es_p", bufs=1) as esp,
                tc.tile_pool(name="osb_p", bufs=1) as osbp,
                tc.tile_pool(name="row_p", bufs=1) as rowp,
                tc.tile_pool(name="cmb_p", bufs=1) as cmbp,
              ):
                ot_acc = otp.tile([128, CIN, NQ], BF16)
                wp_b = wpbp.tile([128, CIN, DIM], BF16)

                def combine_stages(h, osb, zrs):
                    """Two emission stages per (h, qc): stage A ends with the
                    GpSimd all-reduce, stage B starts with the Ln that reads
                    it. Emitting B two attention units later keeps the
                    in-order ACT queue from blocking on the cross-engine
                    round trip (which stalled S matmuls via T-slot WARs)."""
                    p0 = (h % 2) * VD

                    def mkA(qc):
                        zr1, zr2 = zrs[qc]
                        o1 = osb[(0, qc)]
                        o2 = osb[(1, qc)]
                        state = {}

                        def stageA():
                            l1 = rowp.tile([1, QW], F32, tag="l1", bufs=1)
                            nc.scalar.activation(l1, zr1, AF.Ln)
                            l2 = rowp.tile([1, QW], F32, tag="l2", bufs=1)
                            nc.scalar.activation(l2, zr2, AF.Ln)
                            drow = rowp.tile([1, QW], F32, tag="drow", bufs=1)
                            nc.vector.tensor_sub(drow, l1, l2)
                            tr = rowp.tile([1, QW], F32, tag="tr", bufs=1)
                            nc.scalar.activation(tr, drow, AF.Exp)  # Z1/Z2
                            tr2 = rowp.tile([1, QW], BF16, tag="tr2", bufs=1)
                            nc.vector.tensor_scalar_mul(tr2, tr, lam)
                            tb = cmbp.tile([VD, QW], BF16, tag="tb", bufs=1)
                            nc.gpsimd.partition_broadcast(tb, tr2, channels=VD)
                            tmp = cmbp.tile([VD, QW], BF16, tag="tmp", bufs=1)
                            nc.vector.tensor_mul(tmp, tb, o2)
                            od = cmbp.tile([VD, QW], BF16, tag="od", bufs=2)
                            nc.vector.tensor_sub(od, o1, tmp)
                            squ = cmbp.tile([VD + 1, QW], BF16, tag="squ",
                                            bufs=1)
                            nc.vector.tensor_mul(squ[0:VD, :], od, od)
                            ez = rowp.tile([1, QW], F32, tag="ez", bufs=1)
                            nc.vector.tensor_mul(ez, zr1, zr1)
                            nc.vector.tensor_scalar_mul(
                                squ[VD:VD + 1, :], ez, float(VD) * EPS
                            )
                            u = cmbp.tile([VD + 1, QW], F32, tag="u", bufs=2)
                            nc.gpsimd.partition_all_reduce(
                                u, squ, VD + 1, bass_isa.ReduceOp.add
                            )
                            state["od"] = od
                            state["u"] = u

                        def stageB():
                            od = state["od"]
                            u = state["u"]
                            lu = cmbp.tile([VD, QW], F32, tag="lu", bufs=1)
                            nc.scalar.activation(lu, u[0:VD, :], AF.Ln)
                            rr = cmbp.tile([VD, QW], BF16, tag="rr", bufs=1)
                            nc.scalar.activation(rr, lu, AF.Exp, scale=-0.5)
                            on2 = cmbp.tile([VD, QW], BF16, tag="on2", bufs=1)
                            nc.vector.tensor_mul(on2, od, rr)
                            nc.vector.tensor_scalar_mul(
                                ot_acc[p0:p0 + VD, h // 2,
                                       qc * QW:(qc + 1) * QW],
                                on2,
                                weff_t[p0:p0 + VD, :],
                            )

                        return stageA, stageB

                    a0, b0 = mkA(0)
                    a1, b1 = mkA(1)
                    return [a0, a1, b0, b1]

                with (
                    tc.tile_pool(name="Tp", bufs=1, space="PSUM") as Tp,
                    tc.tile_pool(name="psO", bufs=1, space="PSUM") as psOp,
                ):
                    T = Tp.tile([128, 3, 1024], F32)
                    slot_ctr = [0]

                    def attn_unit(qh, kh, vh, qc):
                        o_ps = psOp.tile([65, QW], F32, tag="o")
                        es_info = {}
                        prev = None

                        def emit_pv(kts):
                            for kt in kts:
                                es, half = es_info[kt]
                                for sbi in range(2):
                                    nc.tensor.matmul(
                                        o_ps[:, sbi * 512:(sbi + 1) * 512],
                                        vh[:, kt, 0:VD + 1],
                                        es[:, half, sbi * 512:(sbi + 1) * 512],
                                        start=(kt == 0),
                                        stop=(kt == KT - 1),
                                    )

                        for m in range(KT // 2):
                            cur = []
                            for i in (0, 1):
                                kt = 2 * m + i
                                s = slot_ctr[0]
                                slot_ctr[0] = (s + 1) % 3
                                kb = 64 * (kt % 2)
                                klhs = kh[kb:kb + HD, kt // 2, :]
                                for sbi in range(2):
                                    nc.tensor.matmul(
                                        T[:, s, sbi * 512:(sbi + 1) * 512],
                                        klhs,
                                        qh[kb:kb + HD,
                                           qc * QW + sbi * 512:
                                           qc * QW + (sbi + 1) * 512],
                                        start=True,
                                        stop=True,
                                    )
                                cur.append((kt, s))
                            if prev is not None:
                                emit_pv(prev)
                            (ka, sa), (kb2, sb2) = cur
                            lo, hi = (sa, sb2) if sa < sb2 else (sb2, sa)
                            es = esp.tile([128, 2, 1024], BF16, tag="es", bufs=3)
                            nc.scalar.activation(
                                es, T[:, lo:hi + 1:(hi - lo), :], AF.Exp,
                                scale=SCALE,
                            )
                            es_info[ka] = (es, 0 if sa == lo else 1)
                            es_info[kb2] = (es, 0 if sb2 == lo else 1)
                            prev = (ka, kb2)
                        emit_pv(prev)
                        return o_ps

                    def stage_head(h):
                        vh = stp.tile([128, KT, VD + 2], BF16, tag="vh", bufs=2)
                        nc.sync.dma_start(out=vh[:, :, 0:VD], in_=v_s[h])
                        nc.vector.memset(vh[:, :, VD:VD + 1], 1.0)
                        qks = []
                        for br in range(2):
                            r0 = h * VD + br * HD
                            qh = stp.tile([128, NQ], BF16, tag="qh", bufs=4)
                            for rp in (0, 64):
                                nc.sync.dma_start(
                                    out=qh[rp:rp + HD, :], in_=qt_s[r0:r0 + HD, :]
                                )
                            kh = stp.tile([128, KT // 2, 128], BF16, tag="kh",
                                          bufs=4)
                            for bq in range(2):
                                nc.sync.dma_start(
                                    out=kh[bq * 64:bq * 64 + HD, :, :],
                                    in_=kt_s[r0:r0 + HD, :].rearrange(
                                        "d (g b t) -> d g b t", b=2, t=128
                                    )[:, :, bq, :],
                                )
                            qks.append((qh, kh))
                        return vh, qks

                    # wproj load+convert overlaps head-0 staging/attention
                    for ci in range(CIN):
                        wpf = stp.tile([128, DIM], F32, tag="wpf", bufs=1)
                        nc.sync.dma_start(
                            out=wpf, in_=wprojt[ci * 128:(ci + 1) * 128, :]
                        )
                        nc.vector.tensor_copy(wp_b[:, ci, :], wpf)

                    pending = []
                    staged = stage_head(0)
                    for h in range(H):
                        nxt = stage_head(h + 1) if h + 1 < H else None
                        vh, qks = staged
                        osb = {}
                        zrs = {}
                        for br in range(2):
                            qh, kh = qks[br]
                            for qc in range(2):
                                o_ps = attn_unit(qh, kh, vh, qc)
                                o_sb = osbp.tile([VD, QW], BF16, tag="osb",
                                                 bufs=8)
                                nc.vector.tensor_copy(o_sb, o_ps[0:VD, :])
                                zt = osbp.tile([1, QW], F32, tag=f"zr{br}",
                                               bufs=4)
                                nc.vector.tensor_copy(zt, o_ps[VD:VD + 1, :])
                                if br == 0:
                                    zrs[qc] = [zt, None]
                                else:
                                    zrs[qc][1] = zt
                                osb[(br, qc)] = o_sb
                                # one deferred combine stage per unit boundary
                                if pending:
                                    pending.pop(0)()
                        pending += combine_stages(h, osb, zrs)
                        staged = nxt
                    for st in pending:
                        st()

              # ================= phase C: proj =================
              with (
                  tc.tile_pool(name="psY", bufs=2, space="PSUM") as psY,
                  tc.tile_pool(name="yd_p", bufs=3) as ydp,
              ):
                if True:
                    for qt in range(NQ // 128):
                        yps = psY.tile([128, 1024], F32, tag="y")
                        for sbi in range(2):
                            for ci in range(CIN):
                                nc.tensor.matmul(
                                    yps[:, sbi * 512:(sbi + 1) * 512],
                                    ot_acc[:, ci, qt * 128:(qt + 1) * 128],
                                    wp_b[:, ci, sbi * 512:(sbi + 1) * 512],
                                    start=(ci == 0),
                                    stop=False,
                                )
                            nc.tensor.matmul(
                                yps[:, sbi * 512:(sbi + 1) * 512],
                                ones1,
                                bp_b[:, sbi * 512:(sbi + 1) * 512],
                                start=False,
                                stop=True,
                            )
                        yd = ydp.tile([128, 1024], F32, tag="yd")
                        nc.vector.tensor_copy(yd, yps)
                        nc.sync.dma_start(
                            out=y[qt * 128:(qt + 1) * 128, :], in_=yd
                        )
    nc.finalize()
    return nc


def make_in_maps(inputs):
    x = np.asarray(inputs["x"], np.float32)
    wqkvt = np.ascontiguousarray(np.asarray(inputs["w_qkv"], np.float32).T)
    wprojt = np.ascontiguousarray(np.asarray(inputs["w_proj"], np.float32).T)
    bp = np.asarray(inputs["b_proj"], np.float32).reshape(1, DIM)
    weff = (np.asarray(inputs["sub_norm_w"], np.float32)
            * (1.0 - LAMBDA_INIT) * float(np.sqrt(VD))).reshape(VD, 1)
    in_maps = []
    for c in range(NCORES):
        b, half = c // 2, c % 2
        xt = np.asarray(x[b].T)  # [DIM, N]
        if half == 1:  # query rows first
            xt = np.concatenate([xt[:, NQ:], xt[:, :NQ]], axis=1)
        in_maps.append({
            "xbt": np.ascontiguousarray(xt),
            "wqkvt": wqkvt,
            "wprojt": wprojt,
            "bproj": bp,
            "weff": weff,
        })
    return in_maps


def kernel(x, w_qkv, w_proj, b_proj, lambda_q1, lambda_k1, lambda_q2,
           lambda_k2, sub_norm_w):
    lam = float(
        np.exp(np.sum(np.float64(lambda_q1) * np.float64(lambda_k1)))
        - np.exp(np.sum(np.float64(lambda_q2) * np.float64(lambda_k2)))
        + LAMBDA_INIT
    )

    key = round(lam, 12)
    if key not in _CACHE:
        _CACHE[key] = build_nc(lam)
    nc = _CACHE[key]

    in_maps = make_in_maps(dict(
        x=x, w_qkv=w_qkv, w_proj=w_proj, b_proj=b_proj, sub_norm_w=sub_norm_w
    ))
    res = run_bass_kernel_spmd(nc, in_maps, list(range(NCORES)))
    out = np.empty((B, N, DIM), np.float32)
    for c in range(NCORES):
        b, half = c // 2, c % 2
        out[b, half * NQ:(half + 1) * NQ, :] = res.results[c]["y"]
    return out
